# revision 1
# baseline (speedup 1.0000x reference)
"""Bipartite GNN message-passing layer on 8 Trainium2 NeuronCores.

Strategy (per spec sharding hint): shard target nodes across the 8 cores
(6250 targets/core/direction); partition edges by target so the
scatter-mean is local to each core; replicate source features + weights.

Per core, per direction:
  - edges sorted by target, grouped into 128-target tiles;
  - per-edge source rows gathered HBM->SBUF in bf16 via the Q7 dma_gather
    custom op in 512-index chunks spread over 4 SWDGE queues (descriptor
    generation is the scarce resource; 4 queue contexts overlap it);
    int16 gather indices => edges split by source half (A: src<32768, B:
    the rest, gathered from an offset base);
  - segment-sum runs on the TensorEngine: host-precomputed one-hot fp8
    scatter matrices S (exact in fp8e4) times the gathered bf16 message
    block, accumulated into a PSUM tile per 128 targets;
  - mean+residual fused in one DVE op reading PSUM, PE transpose,
    fp32r dense matmul (+ bias via a K=1 matmul), relu on ACT with
    row-sum accumulation, squared-sum via a DVE fused multiply+accum,
    then a per-direction batched LayerNorm stats pass and a final
    per-tile ACT scale+shift.
"""

import os
import sys

if "/opt/trn_rl_repo" not in sys.path:
    sys.path.insert(0, "/opt/trn_rl_repo")

from contextlib import ExitStack

import ml_dtypes
import numpy as np

D = 256
NNODE = 50000
N_CORES = 8
TPC = NNODE // N_CORES  # targets per core
TILE = 128
NT = (TPC + TILE - 1) // TILE  # 49 target tiles per core
NTP = NT * TILE  # padded target rows per core (6272)
SPLIT = 32768  # int16 index reach; edges split by source half
SUPER = 7  # tiles per supertile (7 x 7 = 49)
NSUP = (NT + SUPER - 1) // SUPER
GCHUNK = 4  # gather call size in 128-edge blocks (512 idxs: fastest mode)
EPS = 1e-5

F8 = ml_dtypes.float8_e4m3
BF16 = ml_dtypes.bfloat16

# test-only hooks (harness leaves these off)
_TRACE = bool(os.environ.get("BGK_TRACE"))
last_result = None

_prog_cache = {}


def _wrap_idx(idx):
    """dma_gather index layout: edge i -> [i % 16, i // 16], replicated
    across the 8 Q7-core partition groups."""
    assert len(idx) % 16 == 0
    w = idx.reshape(-1, 16).T.astype(np.int16)  # [16, n/16]
    return np.tile(w, (8, 1))  # [128, n/16]


def _prep_direction(src, tgt):
    """Host-side shard/sort/pad. Returns (nblk[NT][2] uniform block counts,
    per-core dict of arrays)."""
    deg = np.bincount(tgt, minlength=NNODE).astype(np.float64)
    recip_full = (1.0 / np.maximum(deg, 1.0)).astype(np.float32)

    order = np.argsort(tgt, kind="stable")
    s_all, t_all = src[order], tgt[order]
    cb = np.searchsorted(t_all, np.arange(N_CORES + 1) * TPC)

    segs = []  # [core][tile] -> ((srcA, tgtA), (srcB, tgtB))
    for c in range(N_CORES):
        s = s_all[cb[c] : cb[c + 1]]
        t = t_all[cb[c] : cb[c + 1]] - c * TPC
        tb = np.searchsorted(t, np.arange(NT + 1) * TILE)
        tiles = []
        for ti in range(NT):
            ss = s[tb[ti] : tb[ti + 1]]
            tt = t[tb[ti] : tb[ti + 1]] - ti * TILE
            mA = ss < SPLIT
            tiles.append(((ss[mA], tt[mA]), (ss[~mA] - SPLIT, tt[~mA])))
        segs.append(tiles)

    nblk = np.zeros((NT, 2), np.int64)
    for ti in range(NT):
        for g in range(2):
            mx = max(len(segs[c][ti][g][0]) for c in range(N_CORES))
            nblk[ti, g] = -(-mx // 128)
        if nblk[ti].sum() == 0:
            nblk[ti, 0] = 1
    totblk = int(nblk.sum())
    blk_off = np.concatenate(
        [np.zeros((2, 1), np.int64), np.cumsum(nblk.T, axis=1)], axis=1
    )  # [2, NT+1]

    cores = []
    for c in range(N_CORES):
        idx_cat = [[], []]
        S = np.zeros((128, totblk * 128), np.uint8)
        scol = 0  # S block column counter (A blocks of all tiles first? no:
        # S columns indexed by (g, global block) in blk_off order per group;
        # group A blocks occupy cols [0, blk_off[0,-1}) then B blocks.
        for g in range(2):
            for ti in range(NT):
                n = int(nblk[ti, g]) * 128
                ss, tt = segs[c][ti][g]
                idx = np.zeros(n, np.int64)
                idx[: len(ss)] = ss
                idx_cat[g].append(idx)
                if n:
                    j = np.arange(len(tt))
                    base = int(blk_off[g, ti]) * 128
                    if g == 1:
                        base += int(blk_off[0, NT]) * 128
                    S[j % 128, base + (j // 128) * 128 + tt] = 0x38  # fp8e4 1.0
        idx_g = [_wrap_idx(np.concatenate(idx_cat[g])) for g in range(2)]
        recip = np.zeros(NTP, np.float32)
        recip[:TPC] = recip_full[c * TPC : (c + 1) * TPC]
        cores.append(
            {
                "idxA": idx_g[0],
                "idxB": idx_g[1],
                "S": S.view(F8),
                "recip": recip.reshape(NT, 128).T.copy(),  # [128, NT]
            }
        )
    return nblk, blk_off, totblk, cores


def _build_program(meta_u, meta_i, apply_gamma_beta):
    import concourse.bass as bass
    import concourse.tile as tile
    from concourse import bacc, mybir

    f32 = mybir.dt.float32
    f32r = mybir.dt.float32r
    bf16 = mybir.dt.bfloat16
    f8 = mybir.dt.float8e4
    i16 = mybir.dt.int16
    Alu = mybir.AluOpType
    Act = mybir.ActivationFunctionType

    nc = bacc.Bacc("TRN2", target_bir_lowering=False, debug=False,
                   num_devices=N_CORES, num_swdge_queues=4)

    def din(name, shape, dt):
        return nc.dram_tensor(name, shape, dt, kind="ExternalInput").ap()

    dirs = []
    for d, (nblk, blk_off, totblk) in (("u", meta_u), ("i", meta_i)):
        totA, totB = int(blk_off[0, NT]), int(blk_off[1, NT])
        dirs.append(
            {
                "name": d,
                "nblk": nblk,
                "blk_off": blk_off,
                "totblk": totblk,
                "totg": (totA, totB),
                "src16": din(f"src16_{d}", [NNODE, D], bf16),
                "x": din(f"x_{d}", [NTP, D], f32),
                "W": din(f"W_{d}", [2, 128, D], f32r),
                "bias": din(f"bias_{d}", [1, D], f32r),
                "idx": [
                    din(f"idx0_{d}", [128, totA * 8], i16),
                    din(f"idx1_{d}", [128, totB * 8], i16),
                ],
                "S": din(f"S_{d}", [128, totblk * 128], f8),
                "recip": din(f"recip_{d}", [128, NT], f32),
                "out": nc.dram_tensor(
                    f"out_{d}", [NTP, D], f32, kind="ExternalOutput"
                ).ap(),
            }
        )
    ident_d = din("ident", [128, 128], f32)
    ones_d = din("ones", [1, 128], f32r)
    if apply_gamma_beta:
        gamma_d = din("gamma_rep", [128, D], f32)
        beta_d = din("beta_rep", [128, D], f32)

    qctr = [0]  # SWDGE queue round-robin

    with tile.TileContext(nc) as tc, ExitStack() as ctx:
        consts = ctx.enter_context(tc.tile_pool(name="consts", bufs=1))
        msgs_p = ctx.enter_context(tc.tile_pool(name="msgs", bufs=2))
        s_p = ctx.enter_context(tc.tile_pool(name="sp", bufs=2))
        xio_p = ctx.enter_context(tc.tile_pool(name="xio", bufs=2))
        work = ctx.enter_context(tc.tile_pool(name="work", bufs=3))
        psum_a = ctx.enter_context(tc.tile_pool(name="psa", bufs=3, space="PSUM"))
        psum_t = ctx.enter_context(tc.tile_pool(name="pst", bufs=2, space="PSUM"))
        psum_y = ctx.enter_context(tc.tile_pool(name="psy", bufs=2, space="PSUM"))

        ident_t = consts.tile([128, 128], f32)
        nc.sync.dma_start(ident_t[:], ident_d[:])
        ones_t = consts.tile([1, 128], f32r)
        nc.sync.dma_start(ones_t[:], ones_d[:])
        if apply_gamma_beta:
            gamma_t = consts.tile([128, D], f32)
            nc.sync.dma_start(gamma_t[:], gamma_d[:])
            beta_t = consts.tile([128, D], f32)
            nc.sync.dma_start(beta_t[:], beta_d[:])

        for dd in dirs:
            d = dd["name"]
            nblk, blk_off = dd["nblk"], dd["blk_off"]
            with ExitStack() as dctx:
                dmeta = dctx.enter_context(tc.tile_pool(name=f"meta_{d}", bufs=1))
                W_t = dmeta.tile([128, 2, D], f32r, name=f"W_{d}")
                for h in range(2):
                    nc.sync.dma_start(W_t[:, h, :], dd["W"][h])
                bias_t = dmeta.tile([1, D], f32r, name=f"bias_{d}")
                nc.sync.dma_start(bias_t[:], dd["bias"][:])
                recip_t = dmeta.tile([128, NT], f32, name=f"recip_{d}")
                nc.sync.dma_start(recip_t[:], dd["recip"][:])
                idx_t = []
                for g in range(2):
                    it = dmeta.tile(
                        [128, dd["totg"][g] * 8], i16, name=f"idx{g}_{d}"
                    )
                    nc.sync.dma_start(it[:], dd["idx"][g][:])
                    idx_t.append(it)
                # per-direction arenas
                yr_all = dmeta.tile([128, NT, D], f32, name=f"yr_{d}")
                s1_all = dmeta.tile([128, NT], f32, name=f"s1_{d}")
                s2_all = dmeta.tile([128, NT], f32, name=f"s2_{d}")

                # Software-pipelined emission with a 2-tile skew so the PE
                # instruction stream never waits on a same-tile DVE round
                # trip: A(t) scatter-matmuls, B(t-1) mean+residual+transpose,
                # C(t-2) dense+relu+sq.
                state = {}

                def emit_A(ti, msgs, s_tiles):
                    agg = psum_a.tile([128, D], f32, tag="agg",
                                      name=f"agg_{d}_{ti}")
                    tot_tile_blocks = int(nblk[ti, 0] + nblk[ti, 1])
                    done = 0
                    for g in range(2):
                        nb = int(nblk[ti, g])
                        if nb == 0:
                            continue
                        m, b0 = msgs[g]
                        sb = s_tiles[g]
                        lo = int(blk_off[g, ti]) - b0
                        for k in range(nb):
                            nc.tensor.matmul(
                                agg[:],
                                lhsT=sb[:, lo + k, :],
                                rhs=m[:, lo + k, :],
                                start=(done == 0),
                                stop=(done == tot_tile_blocks - 1),
                            )
                            done += 1
                    state[ti] = {"agg": agg}

                def emit_B(ti, x_sup, tl):
                    st = state[ti]
                    xm = work.tile([128, D], f32, tag="xm", name=f"xm_{d}_{ti}")
                    nc.vector.scalar_tensor_tensor(
                        xm[:], st["agg"][:], recip_t[:, ti : ti + 1],
                        x_sup[:, tl, :], Alu.mult, Alu.add,
                    )
                    tr = psum_t.tile([128, 2, 128], f32, tag="tr",
                                     name=f"tr_{d}_{ti}")
                    for h in range(2):
                        nc.tensor.transpose(
                            tr[:, h, :], xm[:, h * 128 : (h + 1) * 128],
                            ident_t[:],
                        )
                    xmT = work.tile([128, 2, 128], f32r, tag="xmT",
                                    name=f"xmT_{d}_{ti}")
                    nc.vector.tensor_copy(xmT[:], tr[:])
                    st["xmT"] = xmT

                def emit_C(ti):
                    st = state.pop(ti)
                    xmT = st["xmT"]
                    y_ps = psum_y.tile([128, D], f32, tag="y",
                                       name=f"y_{d}_{ti}")
                    nc.tensor.matmul(y_ps[:], lhsT=ones_t[:], rhs=bias_t[:],
                                     start=True, stop=False)
                    for h in range(2):
                        nc.tensor.matmul(
                            y_ps[:], lhsT=xmT[:, h, :], rhs=W_t[:, h, :],
                            start=False, stop=(h == 1),
                        )
                    yr = yr_all[:, ti, :]
                    nc.scalar.activation(
                        yr, y_ps[:], Act.Relu,
                        accum_out=s1_all[:, ti : ti + 1],
                    )
                    sq = work.tile([128, D], f32, tag="sq", name=f"sq_{d}_{ti}")
                    nc.vector.scalar_tensor_tensor(
                        sq[:], yr, 1.0, yr, Alu.mult, Alu.mult,
                        accum_out=s2_all[:, ti : ti + 1],
                    )

                xsup_of = {}
                pend = []  # tiles awaiting B (then C)
                for si in range(NSUP):
                    t0, t1 = si * SUPER, min((si + 1) * SUPER, NT)
                    nts = t1 - t0
                    msgs, s_tiles = [], []
                    for g in range(2):
                        b0, b1 = int(blk_off[g, t0]), int(blk_off[g, t1])
                        nb = b1 - b0
                        if nb == 0:
                            msgs.append((None, b0))
                            s_tiles.append(None)
                            continue
                        m = msgs_p.tile([128, nb, D], bf16, tag=f"msgs{g}",
                                        name=f"msgs{g}_{d}_{si}")
                        src_view = (
                            dd["src16"][0:SPLIT] if g == 0
                            else dd["src16"][SPLIT:NNODE]
                        )
                        for c0 in range(0, nb, GCHUNK):
                            c1 = min(c0 + GCHUNK, nb)
                            nc.gpsimd.dma_gather(
                                m[:, c0:c1, :],
                                src_view,
                                idx_t[g][:, (b0 + c0) * 8 : (b0 + c1) * 8],
                                num_idxs=(c1 - c0) * 128,
                                num_idxs_reg=(c1 - c0) * 128,
                                elem_size=D,
                                single_packet=True,
                                queue_num=qctr[0] % 4,
                            )
                            qctr[0] += 1
                        msgs.append((m, b0))
                        # S slab for this supertile+group
                        sb = s_p.tile([128, nb, 128], f8, tag=f"S{g}",
                                      name=f"S{g}_{d}_{si}")
                        soff = b0 + (dd["totg"][0] if g == 1 else 0)
                        nc.sync.dma_start(
                            sb[:],
                            dd["S"][:, soff * 128 : (soff + nb) * 128],
                        )
                        s_tiles.append(sb)

                    x_sup = xio_p.tile([128, SUPER, D], f32, tag="xs",
                                       name=f"xs_{d}_{si}")
                    nc.sync.dma_start(
                        x_sup[:, :nts, :],
                        dd["x"][t0 * TILE : t1 * TILE].rearrange(
                            "(t p) c -> p t c", p=128
                        ),
                    )
                    for ti in range(t0, t1):
                        xsup_of[ti] = (x_sup, ti - t0)
                        emit_A(ti, msgs, s_tiles)
                        pend.append(ti)
                        if len(pend) >= 2:
                            tb = pend[-2]
                            emit_B(tb, *xsup_of.pop(tb))
                        if len(pend) >= 3:
                            emit_C(pend.pop(0))
                # flush
                if len(pend) >= 1 and pend[-1] in xsup_of:
                    emit_B(pend[-1], *xsup_of.pop(pend[-1]))
                for ti in pend:
                    emit_C(ti)
                pend.clear()

                # batched LN stats for the whole direction
                mu = dmeta.tile([128, NT], f32, name=f"mu_{d}")
                nc.vector.tensor_scalar(mu[:], s1_all[:], 1.0 / D, None, Alu.mult)
                msq = dmeta.tile([128, NT], f32, name=f"msq_{d}")
                nc.vector.tensor_tensor(msq[:], mu[:], mu[:], Alu.mult)
                var = dmeta.tile([128, NT], f32, name=f"var_{d}")
                nc.vector.scalar_tensor_tensor(
                    var[:], s2_all[:], 1.0 / D, msq[:], Alu.mult, Alu.subtract
                )
                veps = dmeta.tile([128, NT], f32, name=f"veps_{d}")
                nc.vector.tensor_scalar(veps[:], var[:], EPS, None, Alu.add)
                rv = dmeta.tile([128, NT], f32, name=f"rv_{d}")
                nc.vector.reciprocal(rv[:], veps[:])
                rstd = dmeta.tile([128, NT], f32, name=f"rstd_{d}")
                nc.scalar.activation(rstd[:], rv[:], Act.Sqrt)
                shift = dmeta.tile([128, NT], f32, name=f"shift_{d}")
                nc.vector.scalar_tensor_tensor(
                    shift[:], mu[:], -1.0, rstd[:], Alu.mult, Alu.mult
                )

                for si in range(NSUP):
                    t0, t1 = si * SUPER, min((si + 1) * SUPER, NT)
                    nts = t1 - t0
                    out_sup = xio_p.tile([128, SUPER, D], f32, tag="os",
                                         name=f"os2_{d}_{si}")
                    for ti in range(t0, t1):
                        tl = ti - t0
                        o = out_sup[:, tl, :]
                        nc.scalar.activation(
                            o, yr_all[:, ti, :], Act.Identity,
                            bias=shift[:, ti : ti + 1],
                            scale=rstd[:, ti : ti + 1],
                        )
                        if apply_gamma_beta:
                            nc.vector.tensor_tensor(o, o, gamma_t[:], Alu.mult)
                            nc.vector.tensor_tensor(o, o, beta_t[:], Alu.add)
                    nc.sync.dma_start(
                        dd["out"][t0 * TILE : t1 * TILE].rearrange(
                            "(t p) c -> p t c", p=128
                        ),
                        out_sup[:, :nts, :],
                    )

    nc.compile()
    return nc


def kernel(
    user_features,
    item_features,
    user_item_edge_index,
    item_user_edge_index,
    Wu,
    bu,
    Wi,
    bi,
    gamma,
    beta,
):
    from concourse.bass_utils import run_bass_kernel_spmd

    uf = np.asarray(user_features, np.float32)
    itf = np.asarray(item_features, np.float32)
    ui = np.asarray(user_item_edge_index)
    iu = np.asarray(item_user_edge_index)
    Wu = np.asarray(Wu, np.float32)
    Wi = np.asarray(Wi, np.float32)
    bu = np.asarray(bu, np.float32)
    bi = np.asarray(bi, np.float32)
    gamma_np = np.asarray(gamma, np.float32)
    beta_np = np.asarray(beta, np.float32)

    # direction "u": targets are users, sources are items
    nblk_u, off_u, tot_u, cores_u = _prep_direction(
        iu[0].astype(np.int64), iu[1].astype(np.int64)
    )
    # direction "i": targets are items, sources are users
    nblk_i, off_i, tot_i, cores_i = _prep_direction(
        ui[0].astype(np.int64), ui[1].astype(np.int64)
    )

    apply_gb = not (np.all(gamma_np == 1.0) and np.all(beta_np == 0.0))

    key = (nblk_u.tobytes(), nblk_i.tobytes(), apply_gb)
    if key not in _prog_cache:
        _prog_cache[key] = _build_program(
            (nblk_u, off_u, tot_u), (nblk_i, off_i, tot_i), apply_gb
        )
    nc = _prog_cache[key]

    ident = np.eye(128, dtype=np.float32)
    ones = np.ones((1, 128), np.float32)
    src16_u = itf.astype(BF16)  # sources for direction u are items
    src16_i = uf.astype(BF16)

    def pad_x(x, c):
        out = np.zeros((NTP, D), np.float32)
        out[:TPC] = x[c * TPC : (c + 1) * TPC]
        return out

    in_maps = []
    for c in range(N_CORES):
        im = {
            "src16_u": src16_u,
            "src16_i": src16_i,
            "x_u": pad_x(uf, c),
            "x_i": pad_x(itf, c),
            "W_u": Wu.reshape(2, 128, D),
            "W_i": Wi.reshape(2, 128, D),
            "bias_u": bu.reshape(1, D),
            "bias_i": bi.reshape(1, D),
            "recip_u": cores_u[c]["recip"],
            "recip_i": cores_i[c]["recip"],
            "idx0_u": cores_u[c]["idxA"],
            "idx1_u": cores_u[c]["idxB"],
            "idx0_i": cores_i[c]["idxA"],
            "idx1_i": cores_i[c]["idxB"],
            "S_u": cores_u[c]["S"],
            "S_i": cores_i[c]["S"],
            "ident": ident,
            "ones": ones,
        }
        if apply_gb:
            im["gamma_rep"] = np.tile(gamma_np[None, :], (128, 1))
            im["beta_rep"] = np.tile(beta_np[None, :], (128, 1))
        in_maps.append(im)

    res = run_bass_kernel_spmd(nc, in_maps, list(range(N_CORES)), trace=_TRACE)
    global last_result
    last_result = res
    u_new = np.concatenate(
        [res.results[c]["out_u"][:TPC] for c in range(N_CORES)]
    )
    i_new = np.concatenate(
        [res.results[c]["out_i"][:TPC] for c in range(N_CORES)]
    )
    return (u_new, i_new)

